# revision 20
# baseline (speedup 1.0000x reference)
"""Trainium2 Bass kernel for nn_AttentionHead (B=4, T=2048, D=1024, H=16).

Math shortcut (exact, validated vs reference):
  pooled[b] = (concat_h[ (w*r_h)^T E_h V_h ] + bv) @ Wo + bo
where E_h = exp(Q_h K_h^T / 8) (no max-subtraction needed: |scores| < ~3),
r = 1/rowsum(E), w[t] = (1/(H*T)) sum_{h,q} E_h[q,t] r_q  (head-avg column
sums of softmax), so the full attn@V [B,H,T,T]x[T,HD] and the [B*T,D]@Wo
matmuls are never materialized.

Sharding: 8 cores = (batch b = core//2) x (head-group g = core%2, 8 heads
each). w mixes all 16 heads of a batch -> one tiny [2048] f32 AllReduce
between core pairs mid-kernel. Host sums the two per-batch partial outputs
and adds the exact bias correction bv@Wo + bo.

Schedule: projections for head-pair dc are interleaved with attention of
heads 2dc,2dc+1 (PE executes in issue order; this keeps ScalarE's exp
stream, the kernel bottleneck, fed). E is spilled to HBM in fp8e4m3 and
re-read in pass 2. All PSUM->SBUF evictions run on VectorE; the score
scale 1/8 is folded into Wq host-side so ScalarE does exps only.
"""

import os
import sys

for _p in ("/opt/trn_rl_repo",):
    if _p not in sys.path and os.path.isdir(_p):
        sys.path.insert(0, _p)

from contextlib import ExitStack

import numpy as np

import concourse.bass as bass
import concourse.mybir as mybir
import concourse.tile as tile
from concourse import bacc
from concourse.bass_utils import run_bass_kernel_spmd
from concourse.masks import make_identity

FP32 = mybir.dt.float32
BF16 = mybir.dt.bfloat16
F8 = mybir.dt.float8e4
AF = mybir.ActivationFunctionType

P = 128
B, T, D, H = 4, 2048, 1024, 16
HD = D // H          # 64
NH = 8               # heads per core
NHD = NH * HD        # 512 cols per core
TQ = T // P          # 16 q-chunks
MC = D // P          # 8 contraction chunks for projections


def _body(tc, x_d, wq_d, wk_d, wv_d, wo_d, bqs_d, bkc_d, out_d,
          single_core=False, dbg=None):
    nc = tc.nc
    with ExitStack() as ctx:
        pers = ctx.enter_context(tc.tile_pool(name="pers", bufs=1))

        def ptile(shape, dtype, name):
            return pers.tile(shape, dtype, name=name, tag=name)

        QT = [ptile([P, T], BF16, f"QT{i}") for i in range(4)]
        KT = [ptile([P, T], BF16, f"KT{i}") for i in range(4)]
        Vt = [ptile([P, NHD], BF16, f"V{i}") for i in range(TQ)]
        wo_bf = [ptile([P, D], BF16, f"wo{i}") for i in range(4)]
        Zh = [ptile([P, 2 * TQ], FP32, f"Z{h}") for h in range(NH)]
        rV = [ptile([P, TQ], FP32, f"rV{h}") for h in range(NH)]
        rB = [ptile([P, TQ], BF16, f"rB{h}") for h in range(NH)]
        gB = [ptile([P, TQ], BF16, f"gB{h}") for h in range(NH)]
        w_col = ptile([P, TQ], FP32, "w_col")
        biasq = ptile([P, 4], FP32, "biasq")
        biask = ptile([P, 4], FP32, "biask")
        ident = ptile([P, P], FP32, "ident")
        zeros_bf = ptile([P, P], BF16, "zeros_bf")
        c_sb = ptile([P, TQ], FP32, "c_sb")

        make_identity(nc, ident)
        nc.gpsimd.memset(zeros_bf, 0.0)
        nc.sync.dma_start(biasq, bqs_d.rearrange("(c p) -> p c", p=P))
        nc.sync.dma_start(biask, bkc_d.rearrange("(c p) -> p c", p=P))

        E2pool = ctx.enter_context(tc.tile_pool(name="E2pool", bufs=14))
        dram = ctx.enter_context(tc.tile_pool(name="dram", bufs=1, space="DRAM"))
        E_spill = dram.tile([NH * T, T], F8, name="E_spill", tag="E_spill")
        c_bounce = dram.tile([1, T], FP32, name="c_bounce", tag="c_bounce")
        w_bounce = dram.tile([1, T], FP32, name="w_bounce", tag="w_bounce")

        with ExitStack() as p1:
            stage = p1.enter_context(tc.tile_pool(name="stage", bufs=4))
            xT = [p1.enter_context(tc.tile_pool(name=f"xTp{m}", bufs=1)).tile(
                [P, T], BF16, name=f"xT{m}", tag=f"xT{m}") for m in range(MC)]
            wq_bf = [p1.enter_context(tc.tile_pool(name=f"wqp{m}", bufs=1)).tile(
                [P, NHD], BF16, name=f"wq{m}", tag=f"wq{m}") for m in range(MC)]
            wk_bf = [p1.enter_context(tc.tile_pool(name=f"wkp{m}", bufs=1)).tile(
                [P, NHD], BF16, name=f"wk{m}", tag=f"wk{m}") for m in range(MC)]
            wv_bf = [p1.enter_context(tc.tile_pool(name=f"wvp{m}", bufs=1)).tile(
                [P, NHD], BF16, name=f"wv{m}", tag=f"wv{m}") for m in range(MC)]
            Epool = p1.enter_context(tc.tile_pool(name="Epool", bufs=6))
            # one [128,1024]-slot pool serves transposes/projections (half
            # used) and score tiles: 2x2 banks; c accumulator: 4 banks.
            psS = p1.enter_context(tc.tile_pool(name="psS", bufs=3, space="PSUM"))
            psC = p1.enter_context(tc.tile_pool(name="psC", bufs=1, space="PSUM"))
            c_ps = psC.tile([P, TQ], FP32, name="c_ps", tag="c_ps")

            # ---- x -> xT (PE transpose, bf16 eviction on DVE) ----
            for grp in range(4):
                xs = []
                for j in range(4):
                    xt = stage.tile([P, D], FP32, name=f"x_{grp}_{j}", tag="x_sb",
                                    bufs=6)
                    nc.sync.dma_start(
                        xt, x_d[(grp * 4 + j) * P:(grp * 4 + j + 1) * P, :])
                    xs.append(xt)
                for m in range(MC):
                    ps = psS.tile([P, 1024], FP32, name=f"trp_{grp}_{m}", tag="S")
                    for j in range(4):
                        nc.tensor.transpose(ps[:, j * P:(j + 1) * P],
                                            xs[j][:, m * P:(m + 1) * P], ident)
                    nc.vector.tensor_copy(xT[m][:, grp * 512:(grp + 1) * 512],
                                          ps[:, :512])

            # ---- weights -> bf16 (wq pre-scaled by 1/8 on host) ----
            for w_d, w_bf, nm in ((wq_d, wq_bf, "q"), (wk_d, wk_bf, "k"),
                                  (wv_d, wv_bf, "v")):
                for m in range(MC):
                    wf = stage.tile([P, NHD], FP32, name=f"wf{nm}{m}", tag="wf32",
                                    bufs=4)
                    nc.sync.dma_start(wf, w_d[m * P:(m + 1) * P, :])
                    nc.vector.tensor_copy(w_bf[m], wf)

            # ---- projections, pipelined one d-chunk ahead of attention;
            # ---- each (Q|K, qq) group split into two 4-matmul sub-bursts
            # ---- so ScalarE's 2-exp PSUM buffer never drains.
            proj_ps = {}

            def proj_sub(dc, sub):
                grp, half = sub // 2, sub % 2
                w_bf, out_t, bias_t = ((wq_bf, QT, biasq) if grp < 4 else
                                       (wk_bf, KT, biask))
                qq = grp % 4
                if half == 0:
                    proj_ps[dc] = psS.tile([P, 1024], FP32,
                                           name=f"pj{dc}_{grp}", tag="S")
                ps = proj_ps[dc]
                for m in range(4 * half, 4 * half + 4):
                    nc.tensor.matmul(ps[:, :512],
                                     lhsT=w_bf[m][:, dc * P:(dc + 1) * P],
                                     rhs=xT[m][:, qq * 512:(qq + 1) * 512],
                                     start=(m == 0), stop=(m == MC - 1))
                if half == 1:
                    nc.vector.tensor_scalar_add(
                        out_t[dc][:, qq * 512:(qq + 1) * 512],
                        ps[:, :512], bias_t[:, dc:dc + 1])

            nc.tensor.matmul(c_ps, lhsT=zeros_bf, rhs=zeros_bf[:, :TQ],
                             start=True, stop=False)

            def c_mms(h, qc, E_sb):
                for kc in range(TQ):
                    nc.tensor.matmul(
                        c_ps[:, kc:kc + 1],
                        lhsT=E_sb[:, kc * P:(kc + 1) * P],
                        rhs=rB[h][:, qc:qc + 1],
                        start=False,
                        stop=(h == NH - 1 and qc == TQ - 1 and kc == TQ - 1))

            prevE = [None]
            e2_pre = {}

            def prefetch_e2(h, qc):
                E2 = E2pool.tile([P, T], F8, name=f"E2_{h}_{qc}", tag="E2")
                nc.sync.dma_start(
                    E2, E_spill[h * T + qc * P:h * T + (qc + 1) * P, :])
                e2_pre[(h, qc)] = E2

            for sub in range(16):
                proj_sub(0, sub)

            for dc in range(4):
                for h in (2 * dc, 2 * dc + 1):
                    ro = (h % 2) * HD
                    for qc in range(TQ):
                        # one projection sub-burst of the NEXT d-chunk per unit
                        if h == 2 * dc and dc < 3:
                            proj_sub(dc + 1, qc)
                        E_sb = Epool.tile([P, T], F8, name=f"E_{h}_{qc}", tag="E")
                        for sh in range(2):
                            ps = psS.tile([P, 1024], FP32,
                                          name=f"S_{h}_{qc}_{sh}", tag="S")
                            for kq in range(2):
                                nc.tensor.matmul(
                                    ps[:, kq * 512:(kq + 1) * 512],
                                    lhsT=QT[dc][ro:ro + HD, qc * P:(qc + 1) * P],
                                    rhs=KT[dc][ro:ro + HD,
                                               sh * 1024 + kq * 512:
                                               sh * 1024 + (kq + 1) * 512],
                                    start=True, stop=True)
                            nc.scalar.activation(
                                E_sb[:, sh * 1024:(sh + 1) * 1024], ps, AF.Exp,
                                accum_out=Zh[h][:, qc * 2 + sh:qc * 2 + sh + 1])
                        nc.sync.dma_start(
                            E_spill[h * T + qc * P:h * T + (qc + 1) * P, :], E_sb)
                        if h == 6 and qc < 14:
                            prefetch_e2(0, qc)
                        # r for this q-chunk, then its c contribution
                        nc.vector.tensor_add(rV[h][:, qc:qc + 1],
                                             Zh[h][:, 2 * qc:2 * qc + 1],
                                             Zh[h][:, 2 * qc + 1:2 * qc + 2])
                        nc.vector.reciprocal(rV[h][:, qc:qc + 1],
                                             rV[h][:, qc:qc + 1])
                        nc.vector.tensor_copy(rB[h][:, qc:qc + 1],
                                              rV[h][:, qc:qc + 1])
                        # c-matmuls lag one q-chunk so PE never waits on exp
                        if qc > 0:
                            c_mms(h, qc - 1, prevE[0])
                        prevE[0] = E_sb
                    c_mms(h, TQ - 1, prevE[0])

            # ---- V (fills the AllReduce bubble) ----
            for ti in range(TQ):
                ps = psS.tile([P, 1024], FP32, name=f"pv{ti}", tag="S")
                for m in range(MC):
                    nc.tensor.matmul(ps[:, :512],
                                     lhsT=xT[m][:, ti * P:(ti + 1) * P],
                                     rhs=wv_bf[m], start=(m == 0),
                                     stop=(m == MC - 1))
                nc.vector.tensor_copy(Vt[ti], ps[:, :512])

            for m in range(4):
                wf = stage.tile([P, D], FP32, name=f"wfo{m}", tag="wof32", bufs=2)
                nc.sync.dma_start(wf, wo_d[m * P:(m + 1) * P, :])
                nc.vector.tensor_copy(wo_bf[m], wf)

            nc.scalar.activation(c_sb, c_ps, AF.Copy, scale=1.0 / (H * T))
            nc.sync.dma_start(
                c_bounce[:].rearrange("a (p c) -> (a p) c", p=P), c_sb)
            if single_core:
                nc.sync.dma_start(w_bounce, c_bounce)
            else:
                nc.gpsimd.collective_compute(
                    "AllReduce", mybir.AluOpType.add,
                    replica_groups=[[0, 1], [2, 3], [4, 5], [6, 7]],
                    ins=[c_bounce[:].opt()], outs=[w_bounce[:].opt()])
            nc.sync.dma_start(
                w_col, w_bounce[:].rearrange("a (p c) -> (a p) c", p=P))

        # -------- pass 2: uT = E^T(w*r) via E-stationary matmuls, ------
        # -------- pooledT = V^T u via V-stationary, then @ Wo ----------
        with ExitStack() as p2:
            small = p2.enter_context(tc.tile_pool(name="small", bufs=2))
            psU = p2.enter_context(tc.tile_pool(name="psU", bufs=2, space="PSUM"))
            psP = p2.enter_context(tc.tile_pool(name="psP", bufs=1, space="PSUM"))
            pooledT_ps = psP.tile([P, 4], FP32, name="pooledT_ps",
                                  tag="pooledT_ps")
            nc.tensor.matmul(pooledT_ps, lhsT=zeros_bf, rhs=zeros_bf[:, :4],
                             start=True, stop=False)

            for h in range(NH):
                gf = small.tile([P, TQ], FP32, name=f"gf{h}", tag="gf")
                nc.vector.tensor_mul(gf, w_col, rV[h])
                nc.vector.tensor_copy(gB[h], gf)

            def pooled_mms(h, u_bf):
                ro, co = (h % 2) * HD, h // 2
                for kc in range(TQ):
                    nc.tensor.matmul(pooledT_ps[ro:ro + HD, co:co + 1],
                                     lhsT=Vt[kc][:, h * HD:(h + 1) * HD],
                                     rhs=u_bf[:, kc:kc + 1],
                                     start=False,
                                     stop=(kc == TQ - 1 and h == NH - 1))

            prev_u = [None]
            dbg_u_t = [None]
            for h in range(NH):
                u_ps = psU.tile([P, TQ], FP32, name=f"u_ps{h}", tag="u_ps")
                nc.tensor.matmul(u_ps, lhsT=zeros_bf, rhs=zeros_bf[:, :TQ],
                                 start=True, stop=False)
                for qc in range(TQ):
                    E2 = e2_pre.pop((h, qc), None)
                    if E2 is None:
                        E2 = E2pool.tile([P, T], F8, name=f"E2_{h}_{qc}", tag="E2")
                        nc.sync.dma_start(
                            E2, E_spill[h * T + qc * P:h * T + (qc + 1) * P, :])
                    for kc in range(TQ):
                        nc.tensor.matmul(u_ps[:, kc:kc + 1],
                                         lhsT=E2[:, kc * P:(kc + 1) * P],
                                         rhs=gB[h][:, qc:qc + 1],
                                         start=False,
                                         stop=(qc == TQ - 1 and kc == TQ - 1))
                u_bf = small.tile([P, TQ], BF16, name=f"u_bf{h}",
                                  tag="u_bf0" if h == 0 else "u_bf",
                                  bufs=1 if h == 0 else 3)
                if h == 0:
                    dbg_u_t[0] = u_bf
                nc.vector.tensor_copy(u_bf, u_ps)
                if prev_u[0] is not None:
                    pooled_mms(h - 1, prev_u[0])
                prev_u[0] = u_bf
            pooled_mms(NH - 1, prev_u[0])

            pooledT_bf = small.tile([P, 4], BF16, name="pooledT_bf",
                                    tag="pooledT_bf")
            nc.vector.tensor_copy(pooledT_bf, pooledT_ps)

            part_ps = psU.tile([1, D], FP32, name="part_ps", tag="part_ps")
            for mc in range(4):
                for hf in range(2):
                    nc.tensor.matmul(part_ps[0:1, hf * 512:(hf + 1) * 512],
                                     lhsT=pooledT_bf[:, mc:mc + 1],
                                     rhs=wo_bf[mc][:, hf * 512:(hf + 1) * 512],
                                     start=(mc == 0), stop=(mc == 3))
            out_sb = small.tile([1, D], FP32, name="out_sb", tag="out_sb")
            nc.vector.tensor_copy(out_sb, part_ps)
            nc.sync.dma_start(out_d[:], out_sb)
            for nm, src_t in (("dbg_w", w_col), ("dbg_r", rV[0]),
                              ("dbg_u", dbg_u_t[0]), ("dbg_p", pooledT_bf),
                              ("dbg_c", c_sb)):
                nc.sync.dma_start(dbg[nm], src_t)


_NC_CACHE = {}


def build_nc(single_core=False):
    if single_core in _NC_CACHE:
        return _NC_CACHE[single_core]
    nc = bacc.Bacc("TRN2", target_bir_lowering=False, debug=False,
                   enable_asserts=False, num_devices=1 if single_core else 8)
    x_d = nc.dram_tensor("x", [T, D], FP32, kind="ExternalInput")
    wq_d = nc.dram_tensor("wq", [D, NHD], FP32, kind="ExternalInput")
    wk_d = nc.dram_tensor("wk", [D, NHD], FP32, kind="ExternalInput")
    wv_d = nc.dram_tensor("wv", [D, NHD], FP32, kind="ExternalInput")
    wo_d = nc.dram_tensor("wo", [NHD, D], FP32, kind="ExternalInput")
    bqs_d = nc.dram_tensor("bqs", [NHD], FP32, kind="ExternalInput")
    bkc_d = nc.dram_tensor("bkc", [NHD], FP32, kind="ExternalInput")
    out_d = nc.dram_tensor("out", [1, D], FP32, kind="ExternalOutput")
    dbg = {nm: nc.dram_tensor(nm, shp, dt, kind="ExternalOutput").ap()
           for nm, shp, dt in (("dbg_w", [P, TQ], FP32),
                               ("dbg_r", [P, TQ], FP32),
                               ("dbg_u", [P, TQ], BF16),
                               ("dbg_p", [P, 4], BF16),
                               ("dbg_c", [P, TQ], FP32))}
    with tile.TileContext(nc) as tc:
        _body(tc, x_d.ap(), wq_d.ap(), wk_d.ap(), wv_d.ap(), wo_d.ap(),
              bqs_d.ap(), bkc_d.ap(), out_d.ap(), single_core=single_core,
              dbg=dbg)
    nc.compile()
    _NC_CACHE[single_core] = nc
    return nc


def make_in_maps(x, Wq, bq, Wk, bk, Wv, bv, Wo, bo):
    in_maps = []
    for core in range(8):
        b, g = core // 2, core % 2
        cs = slice(g * NHD, (g + 1) * NHD)
        in_maps.append({
            "x": np.ascontiguousarray(x[b]),
            "wq": np.ascontiguousarray(Wq[:, cs]) * np.float32(0.125),
            "wk": np.ascontiguousarray(Wk[:, cs]),
            "wv": np.ascontiguousarray(Wv[:, cs]),
            "wo": np.ascontiguousarray(Wo[cs, :]),
            "bqs": np.ascontiguousarray(bq[cs]) * np.float32(0.125),
            "bkc": np.ascontiguousarray(bk[cs]),
        })
    return in_maps


def kernel(x, Wq, bq, Wk, bk, Wv, bv, Wo, bo, _results_hook=None):
    x, Wq, bq, Wk, bk, Wv, bv, Wo, bo = (
        np.asarray(a, dtype=np.float32)
        for a in (x, Wq, bq, Wk, bk, Wv, bv, Wo, bo))
    nc = build_nc()
    in_maps = make_in_maps(x, Wq, bq, Wk, bk, Wv, bv, Wo, bo)
    res = run_bass_kernel_spmd(nc, in_maps, core_ids=list(range(8)))
    if _results_hook is not None:
        _results_hook(res)
    parts = [res.results[c]["out"][0] for c in range(8)]
    correction = bv.astype(np.float32) @ Wo.astype(np.float32) + bo
    out = np.stack([parts[2 * b] + parts[2 * b + 1] for b in range(B)])
    return (out + correction[None, :]).astype(np.float32)


# revision 22
# speedup vs baseline: 1.0002x; 1.0002x over previous
"""Trainium2 Bass kernel for nn_AttentionHead (B=4, T=2048, D=1024, H=16).

Math shortcut (exact, validated vs reference):
  pooled[b] = (concat_h[ (w*r_h)^T E_h V_h ] + bv) @ Wo + bo
where E_h = exp(Q_h K_h^T / 8) (no max-subtraction needed: |scores| < ~3),
r = 1/rowsum(E), w[t] = (1/(H*T)) sum_{h,q} E_h[q,t] r_q  (head-avg column
sums of softmax), so the full attn@V [B,H,T,T]x[T,HD] and the [B*T,D]@Wo
matmuls are never materialized.

Sharding: 8 cores = (batch b = core//2) x (head-group g = core%2, 8 heads
each). w mixes all 16 heads of a batch -> one tiny [2048] f32 AllReduce
between core pairs mid-kernel. Host sums the two per-batch partial outputs
and adds the exact bias correction bv@Wo + bo.

Schedule: projections for head-pair dc are interleaved with attention of
heads 2dc,2dc+1 (PE executes in issue order; this keeps ScalarE's exp
stream, the kernel bottleneck, fed). E is spilled to HBM in fp8e4m3 and
re-read in pass 2. All PSUM->SBUF evictions run on VectorE; the score
scale 1/8 is folded into Wq host-side so ScalarE does exps only.
"""

import os
import sys

for _p in ("/opt/trn_rl_repo",):
    if _p not in sys.path and os.path.isdir(_p):
        sys.path.insert(0, _p)

from contextlib import ExitStack

import numpy as np

import concourse.bass as bass
import concourse.mybir as mybir
import concourse.tile as tile
from concourse import bacc
from concourse.bass_utils import run_bass_kernel_spmd
from concourse.masks import make_identity

FP32 = mybir.dt.float32
BF16 = mybir.dt.bfloat16
F8 = mybir.dt.bfloat16  # TEMP: was float8e4
AF = mybir.ActivationFunctionType

P = 128
B, T, D, H = 4, 2048, 1024, 16
HD = D // H          # 64
NH = 8               # heads per core
NHD = NH * HD        # 512 cols per core
TQ = T // P          # 16 q-chunks
MC = D // P          # 8 contraction chunks for projections


def _body(tc, x_d, wq_d, wk_d, wv_d, wo_d, bqs_d, bkc_d, out_d,
          single_core=False, dbg=None):
    nc = tc.nc
    with ExitStack() as ctx:
        pers = ctx.enter_context(tc.tile_pool(name="pers", bufs=1))

        def ptile(shape, dtype, name):
            return pers.tile(shape, dtype, name=name, tag=name)

        QT = [ptile([P, T], BF16, f"QT{i}") for i in range(4)]
        KT = [ptile([P, T], BF16, f"KT{i}") for i in range(4)]
        Vt = [ptile([P, NHD], BF16, f"V{i}") for i in range(TQ)]
        wo_bf = [ptile([P, D], BF16, f"wo{i}") for i in range(4)]
        Zh = [ptile([P, 2 * TQ], FP32, f"Z{h}") for h in range(NH)]
        rV = [ptile([P, TQ], FP32, f"rV{h}") for h in range(NH)]
        rB = [ptile([P, TQ], BF16, f"rB{h}") for h in range(NH)]
        gB = [ptile([P, TQ], BF16, f"gB{h}") for h in range(NH)]
        w_col = ptile([P, TQ], FP32, "w_col")
        biasq = ptile([P, 4], FP32, "biasq")
        biask = ptile([P, 4], FP32, "biask")
        ident = ptile([P, P], FP32, "ident")
        zeros_bf = ptile([P, P], BF16, "zeros_bf")
        c_sb = ptile([P, TQ], FP32, "c_sb")

        make_identity(nc, ident)
        nc.gpsimd.memset(zeros_bf, 0.0)
        nc.sync.dma_start(biasq, bqs_d.rearrange("(c p) -> p c", p=P))
        nc.sync.dma_start(biask, bkc_d.rearrange("(c p) -> p c", p=P))

        E2pool = ctx.enter_context(tc.tile_pool(name="E2pool", bufs=8))
        dram = ctx.enter_context(tc.tile_pool(name="dram", bufs=1, space="DRAM"))
        E_spill = dram.tile([NH * T, T], F8, name="E_spill", tag="E_spill")
        c_bounce = dram.tile([1, T], FP32, name="c_bounce", tag="c_bounce")
        w_bounce = dram.tile([1, T], FP32, name="w_bounce", tag="w_bounce")

        with ExitStack() as p1:
            stage = p1.enter_context(tc.tile_pool(name="stage", bufs=4))
            xT = [p1.enter_context(tc.tile_pool(name=f"xTp{m}", bufs=1)).tile(
                [P, T], BF16, name=f"xT{m}", tag=f"xT{m}") for m in range(MC)]
            wq_bf = [p1.enter_context(tc.tile_pool(name=f"wqp{m}", bufs=1)).tile(
                [P, NHD], BF16, name=f"wq{m}", tag=f"wq{m}") for m in range(MC)]
            wk_bf = [p1.enter_context(tc.tile_pool(name=f"wkp{m}", bufs=1)).tile(
                [P, NHD], BF16, name=f"wk{m}", tag=f"wk{m}") for m in range(MC)]
            wv_bf = [p1.enter_context(tc.tile_pool(name=f"wvp{m}", bufs=1)).tile(
                [P, NHD], BF16, name=f"wv{m}", tag=f"wv{m}") for m in range(MC)]
            Epool = p1.enter_context(tc.tile_pool(name="Epool", bufs=5))
            # one [128,1024]-slot pool serves transposes/projections (half
            # used) and score tiles: 2x2 banks; c accumulator: 4 banks.
            psS = p1.enter_context(tc.tile_pool(name="psS", bufs=3, space="PSUM"))
            psC = p1.enter_context(tc.tile_pool(name="psC", bufs=1, space="PSUM"))
            c_ps = psC.tile([P, TQ], FP32, name="c_ps", tag="c_ps")

            # ---- x -> xT (PE transpose, bf16 eviction on DVE) ----
            for grp in range(4):
                xs = []
                for j in range(4):
                    xt = stage.tile([P, D], FP32, name=f"x_{grp}_{j}", tag="x_sb",
                                    bufs=6)
                    nc.sync.dma_start(
                        xt, x_d[(grp * 4 + j) * P:(grp * 4 + j + 1) * P, :])
                    xs.append(xt)
                for m in range(MC):
                    ps = psS.tile([P, 1024], FP32, name=f"trp_{grp}_{m}", tag="S")
                    for j in range(4):
                        nc.tensor.transpose(ps[:, j * P:(j + 1) * P],
                                            xs[j][:, m * P:(m + 1) * P], ident)
                    nc.vector.tensor_copy(xT[m][:, grp * 512:(grp + 1) * 512],
                                          ps[:, :512])

            # ---- weights -> bf16 (wq pre-scaled by 1/8 on host) ----
            for w_d, w_bf, nm in ((wq_d, wq_bf, "q"), (wk_d, wk_bf, "k"),
                                  (wv_d, wv_bf, "v")):
                for m in range(MC):
                    wf = stage.tile([P, NHD], FP32, name=f"wf{nm}{m}", tag="wf32",
                                    bufs=4)
                    nc.sync.dma_start(wf, w_d[m * P:(m + 1) * P, :])
                    nc.vector.tensor_copy(w_bf[m], wf)

            # ---- projections, pipelined one d-chunk ahead of attention;
            # ---- each (Q|K, qq) group split into two 4-matmul sub-bursts
            # ---- so ScalarE's 2-exp PSUM buffer never drains.
            proj_ps = {}

            def proj_sub(dc, sub):
                grp, half = sub // 2, sub % 2
                w_bf, out_t, bias_t = ((wq_bf, QT, biasq) if grp < 4 else
                                       (wk_bf, KT, biask))
                qq = grp % 4
                if half == 0:
                    proj_ps[dc] = psS.tile([P, 1024], FP32,
                                           name=f"pj{dc}_{grp}", tag="S")
                ps = proj_ps[dc]
                for m in range(4 * half, 4 * half + 4):
                    nc.tensor.matmul(ps[:, :512],
                                     lhsT=w_bf[m][:, dc * P:(dc + 1) * P],
                                     rhs=xT[m][:, qq * 512:(qq + 1) * 512],
                                     start=(m == 0), stop=(m == MC - 1))
                if half == 1:
                    nc.vector.tensor_scalar_add(
                        out_t[dc][:, qq * 512:(qq + 1) * 512],
                        ps[:, :512], bias_t[:, dc:dc + 1])

            nc.tensor.matmul(c_ps, lhsT=zeros_bf, rhs=zeros_bf[:, :TQ],
                             start=True, stop=False)

            def c_mms(h, qc, E_sb):
                for kc in range(TQ):
                    nc.tensor.matmul(
                        c_ps[:, kc:kc + 1],
                        lhsT=E_sb[:, kc * P:(kc + 1) * P],
                        rhs=rB[h][:, qc:qc + 1],
                        start=False,
                        stop=(h == NH - 1 and qc == TQ - 1 and kc == TQ - 1))

            prevE = [None]
            e2_pre = {}

            def prefetch_e2(h, qc):
                E2 = E2pool.tile([P, T], F8, name=f"E2_{h}_{qc}", tag="E2")
                nc.sync.dma_start(
                    E2, E_spill[h * T + qc * P:h * T + (qc + 1) * P, :])
                e2_pre[(h, qc)] = E2

            for sub in range(16):
                proj_sub(0, sub)

            for dc in range(4):
                for h in (2 * dc, 2 * dc + 1):
                    ro = (h % 2) * HD
                    for qc in range(TQ):
                        # one projection sub-burst of the NEXT d-chunk per unit
                        if h == 2 * dc and dc < 3:
                            proj_sub(dc + 1, qc)
                        E_sb = Epool.tile([P, T], F8, name=f"E_{h}_{qc}", tag="E")
                        for sh in range(2):
                            ps = psS.tile([P, 1024], FP32,
                                          name=f"S_{h}_{qc}_{sh}", tag="S")
                            for kq in range(2):
                                nc.tensor.matmul(
                                    ps[:, kq * 512:(kq + 1) * 512],
                                    lhsT=QT[dc][ro:ro + HD, qc * P:(qc + 1) * P],
                                    rhs=KT[dc][ro:ro + HD,
                                               sh * 1024 + kq * 512:
                                               sh * 1024 + (kq + 1) * 512],
                                    start=True, stop=True)
                            nc.scalar.activation(
                                E_sb[:, sh * 1024:(sh + 1) * 1024], ps, AF.Exp,
                                accum_out=Zh[h][:, qc * 2 + sh:qc * 2 + sh + 1])
                        nc.sync.dma_start(
                            E_spill[h * T + qc * P:h * T + (qc + 1) * P, :], E_sb)
                        if h == 6 and qc < 14:
                            prefetch_e2(0, qc)
                        # r for this q-chunk, then its c contribution
                        nc.vector.tensor_add(rV[h][:, qc:qc + 1],
                                             Zh[h][:, 2 * qc:2 * qc + 1],
                                             Zh[h][:, 2 * qc + 1:2 * qc + 2])
                        nc.vector.reciprocal(rV[h][:, qc:qc + 1],
                                             rV[h][:, qc:qc + 1])
                        nc.vector.tensor_copy(rB[h][:, qc:qc + 1],
                                              rV[h][:, qc:qc + 1])
                        # c-matmuls lag one q-chunk so PE never waits on exp
                        if qc > 0:
                            c_mms(h, qc - 1, prevE[0])
                        prevE[0] = E_sb
                    c_mms(h, TQ - 1, prevE[0])

            # ---- V (fills the AllReduce bubble) ----
            for ti in range(TQ):
                ps = psS.tile([P, 1024], FP32, name=f"pv{ti}", tag="S")
                for m in range(MC):
                    nc.tensor.matmul(ps[:, :512],
                                     lhsT=xT[m][:, ti * P:(ti + 1) * P],
                                     rhs=wv_bf[m], start=(m == 0),
                                     stop=(m == MC - 1))
                nc.vector.tensor_copy(Vt[ti], ps[:, :512])

            for m in range(4):
                wf = stage.tile([P, D], FP32, name=f"wfo{m}", tag="wof32", bufs=2)
                nc.sync.dma_start(wf, wo_d[m * P:(m + 1) * P, :])
                nc.vector.tensor_copy(wo_bf[m], wf)

            nc.scalar.activation(c_sb, c_ps, AF.Copy, scale=1.0 / (H * T))
            nc.sync.dma_start(
                c_bounce[:].rearrange("a (p c) -> (a p) c", p=P), c_sb)
            if single_core:
                nc.sync.dma_start(w_bounce, c_bounce)
            else:
                nc.gpsimd.collective_compute(
                    "AllReduce", mybir.AluOpType.add,
                    replica_groups=[[0, 1], [2, 3], [4, 5], [6, 7]],
                    ins=[c_bounce[:].opt()], outs=[w_bounce[:].opt()])
            nc.sync.dma_start(
                w_col, w_bounce[:].rearrange("a (p c) -> (a p) c", p=P))

        # -------- pass 2: uT = E^T(w*r) via E-stationary matmuls, ------
        # -------- pooledT = V^T u via V-stationary, then @ Wo ----------
        with ExitStack() as p2:
            small = p2.enter_context(tc.tile_pool(name="small", bufs=2))
            psU = p2.enter_context(tc.tile_pool(name="psU", bufs=2, space="PSUM"))
            psP = p2.enter_context(tc.tile_pool(name="psP", bufs=1, space="PSUM"))
            pooledT_ps = psP.tile([P, 4], FP32, name="pooledT_ps",
                                  tag="pooledT_ps")
            nc.tensor.matmul(pooledT_ps, lhsT=zeros_bf, rhs=zeros_bf[:, :4],
                             start=True, stop=False)

            for h in range(NH):
                gf = small.tile([P, TQ], FP32, name=f"gf{h}", tag="gf")
                nc.vector.tensor_mul(gf, w_col, rV[h])
                nc.vector.tensor_copy(gB[h], gf)

            def pooled_mms(h, u_bf):
                ro, co = (h % 2) * HD, h // 2
                for kc in range(TQ):
                    nc.tensor.matmul(pooledT_ps[ro:ro + HD, co:co + 1],
                                     lhsT=Vt[kc][:, h * HD:(h + 1) * HD],
                                     rhs=u_bf[:, kc:kc + 1],
                                     start=False,
                                     stop=(kc == TQ - 1 and h == NH - 1))

            prev_u = [None]
            dbg_u_t = [None]
            for h in range(NH):
                u_ps = psU.tile([P, TQ], FP32, name=f"u_ps{h}", tag="u_ps")
                nc.tensor.matmul(u_ps, lhsT=zeros_bf, rhs=zeros_bf[:, :TQ],
                                 start=True, stop=False)
                for qc in range(TQ):
                    E2 = e2_pre.pop((h, qc), None)
                    if E2 is None:
                        E2 = E2pool.tile([P, T], F8, name=f"E2_{h}_{qc}", tag="E2")
                        nc.sync.dma_start(
                            E2, E_spill[h * T + qc * P:h * T + (qc + 1) * P, :])
                    for kc in range(TQ):
                        nc.tensor.matmul(u_ps[:, kc:kc + 1],
                                         lhsT=E2[:, kc * P:(kc + 1) * P],
                                         rhs=gB[h][:, qc:qc + 1],
                                         start=False,
                                         stop=(qc == TQ - 1 and kc == TQ - 1))
                u_bf = small.tile([P, TQ], BF16, name=f"u_bf{h}",
                                  tag="u_bf0" if h == 0 else "u_bf",
                                  bufs=1 if h == 0 else 3)
                if h == 0:
                    dbg_u_t[0] = u_bf
                nc.vector.tensor_copy(u_bf, u_ps)
                if prev_u[0] is not None:
                    pooled_mms(h - 1, prev_u[0])
                prev_u[0] = u_bf
            pooled_mms(NH - 1, prev_u[0])

            pooledT_bf = small.tile([P, 4], BF16, name="pooledT_bf",
                                    tag="pooledT_bf")
            nc.vector.tensor_copy(pooledT_bf, pooledT_ps)

            part_ps = psU.tile([1, D], FP32, name="part_ps", tag="part_ps")
            for mc in range(4):
                for hf in range(2):
                    nc.tensor.matmul(part_ps[0:1, hf * 512:(hf + 1) * 512],
                                     lhsT=pooledT_bf[:, mc:mc + 1],
                                     rhs=wo_bf[mc][:, hf * 512:(hf + 1) * 512],
                                     start=(mc == 0), stop=(mc == 3))
            out_sb = small.tile([1, D], FP32, name="out_sb", tag="out_sb")
            nc.vector.tensor_copy(out_sb, part_ps)
            nc.sync.dma_start(out_d[:], out_sb)
            for nm, src_t in (("dbg_w", w_col), ("dbg_r", rV[0]),
                              ("dbg_u", dbg_u_t[0]), ("dbg_p", pooledT_bf),
                              ("dbg_c", c_sb)):
                nc.sync.dma_start(dbg[nm], src_t)


_NC_CACHE = {}


def build_nc(single_core=False):
    if single_core in _NC_CACHE:
        return _NC_CACHE[single_core]
    nc = bacc.Bacc("TRN2", target_bir_lowering=False, debug=False,
                   enable_asserts=False, num_devices=1 if single_core else 8)
    x_d = nc.dram_tensor("x", [T, D], FP32, kind="ExternalInput")
    wq_d = nc.dram_tensor("wq", [D, NHD], FP32, kind="ExternalInput")
    wk_d = nc.dram_tensor("wk", [D, NHD], FP32, kind="ExternalInput")
    wv_d = nc.dram_tensor("wv", [D, NHD], FP32, kind="ExternalInput")
    wo_d = nc.dram_tensor("wo", [NHD, D], FP32, kind="ExternalInput")
    bqs_d = nc.dram_tensor("bqs", [NHD], FP32, kind="ExternalInput")
    bkc_d = nc.dram_tensor("bkc", [NHD], FP32, kind="ExternalInput")
    out_d = nc.dram_tensor("out", [1, D], FP32, kind="ExternalOutput")
    dbg = {nm: nc.dram_tensor(nm, shp, dt, kind="ExternalOutput").ap()
           for nm, shp, dt in (("dbg_w", [P, TQ], FP32),
                               ("dbg_r", [P, TQ], FP32),
                               ("dbg_u", [P, TQ], BF16),
                               ("dbg_p", [P, 4], BF16),
                               ("dbg_c", [P, TQ], FP32))}
    with tile.TileContext(nc) as tc:
        _body(tc, x_d.ap(), wq_d.ap(), wk_d.ap(), wv_d.ap(), wo_d.ap(),
              bqs_d.ap(), bkc_d.ap(), out_d.ap(), single_core=single_core,
              dbg=dbg)
    nc.compile()
    _NC_CACHE[single_core] = nc
    return nc


def make_in_maps(x, Wq, bq, Wk, bk, Wv, bv, Wo, bo):
    in_maps = []
    for core in range(8):
        b, g = core // 2, core % 2
        cs = slice(g * NHD, (g + 1) * NHD)
        in_maps.append({
            "x": np.ascontiguousarray(x[b]),
            "wq": np.ascontiguousarray(Wq[:, cs]) * np.float32(0.125),
            "wk": np.ascontiguousarray(Wk[:, cs]),
            "wv": np.ascontiguousarray(Wv[:, cs]),
            "wo": np.ascontiguousarray(Wo[cs, :]),
            "bqs": np.ascontiguousarray(bq[cs]) * np.float32(0.125),
            "bkc": np.ascontiguousarray(bk[cs]),
        })
    return in_maps


def kernel(x, Wq, bq, Wk, bk, Wv, bv, Wo, bo, _results_hook=None):
    x, Wq, bq, Wk, bk, Wv, bv, Wo, bo = (
        np.asarray(a, dtype=np.float32)
        for a in (x, Wq, bq, Wk, bk, Wv, bv, Wo, bo))
    nc = build_nc()
    in_maps = make_in_maps(x, Wq, bq, Wk, bk, Wv, bv, Wo, bo)
    res = run_bass_kernel_spmd(nc, in_maps, core_ids=list(range(8)))
    if _results_hook is not None:
        _results_hook(res)
    parts = [res.results[c]["out"][0] for c in range(8)]
    correction = bv.astype(np.float32) @ Wo.astype(np.float32) + bo
    out = np.stack([parts[2 * b] + parts[2 * b + 1] for b in range(B)])
    return (out + correction[None, :]).astype(np.float32)
